# revision 9
# baseline (speedup 1.0000x reference)
"""Bass/Trainium2 kernel for the decomposed LocallyConnected2d layer.

out[b,o,i,j] = sum_{c,k} x[b, c, i+di, j+dj] * w[o, c, i, j, k] + bias[o,i,j]
with k = di*3 + dj (3x3 kernel, stride 1).

Strategy: shard over output rows i across 8 cores (4 rows each). Each core
owns 1/8 of the per-location weight (the dominant traffic) and a 6-row halo
slice of x. Per output location (i,j) the contraction (c,k)=288 is split into
3 chunks of 96 = (c,di) indexed, chunked over dj; each chunk is one matmul
lhsT=[96,64] rhs=[96,128] accumulating into PSUM [64 o, 128 b]. Even/odd j
use PE column groups 0/1 (tile_position) so two locations' matmuls overlap
in the array. All matmul data is fp16 (fp32 accumulate in PSUM); output is
written fp16 and upcast on the host. Bias is added on the host in fp32
(cheaper than a 97th contraction row; removes the ones/bias DMAs).

The kernel is HBM-read-bound: per-core inbound is w 4.72 MB + x 3.34 MB
(dup factor 2: partition (c,di) keeps rows i+di as one shared 4-row window)
at a measured ~230 GB/s for 96-partition reads. Scheduling is therefore
everything: ALL inbound DMAs ride ONE HWDGE ring (sync) in output-row order
(xs0, w-row0, xsr-row1, w-row1, ...) so row i's matmuls unblock after
(i+1)/4 of the stream instead of after all of it; every transfer keeps
4-8.7 KB contiguous runs per partition (large packets starve the other
queue, tiny ones pay per-packet overhead). Output DMAs (HBM writes run
~1.8x faster per engine than reads) ride the otherwise-idle scalar ring at
half-row granularity so the write tail after the last matmul is short.
PSUM->SBUF copies alternate Vector/Scalar engines.
"""

import sys

for _p in ("/opt/trn_rl_repo", "/root/.axon_site/_ro/trn_rl_repo"):
    if _p not in sys.path:
        sys.path.append(_p)

import numpy as np

B = 128
C_IN = 32
C_OUT = 64
OH = OW = 32
KH = KW = 3
H = W = 34
N_CORES = 8
RPC = OH // N_CORES          # output rows per core = 4
HALO = RPC + KH - 1          # x rows per core = 6
NPAIR = OW // 2              # j-pairs per row = 16
NGRP = 4                     # j-pairs per psum group
GRPS = NPAIR // NGRP         # psum groups per row = 4

_DT_MM = "float16"
_DT_OUT = "float16"

_prog_cache = {}


def _build_program():
    import concourse.tile as tile
    from concourse import bacc, mybir
    from bass_rust import AP

    dt_mm = getattr(mybir.dt, _DT_MM)
    dt_out = getattr(mybir.dt, _DT_OUT)
    f32 = mybir.dt.float32
    f8 = mybir.dt.float8e4

    nc = bacc.Bacc("TRN2", target_bir_lowering=False, debug=False,
                   num_devices=N_CORES)

    # Per-core DRAM I/O (host pre-sharded / pre-transposed, fp16):
    #   x_in [c=32, h=6, w=34, b=128]  halo slice, b innermost
    #   w_in [r=288, i=4, j=32, o=64]  r = dj*96 + c*3 + di
    #   out  [p2=128 (par*64+o), i=4, jh=16, b=128] ; j = 2*jh + par
    x_in = nc.dram_tensor("x", [C_IN, HALO, W, B], dt_mm,
                          kind="ExternalInput").ap()
    w16_in = nc.dram_tensor("w16", [96, RPC, OW, C_OUT], dt_mm,
                            kind="ExternalInput").ap()
    w8_in = nc.dram_tensor("w8", [192, RPC, OW, C_OUT], f8,
                           kind="ExternalInput").ap()
    out = nc.dram_tensor("out", [B, RPC, NPAIR, B], dt_out,
                         kind="ExternalOutput").ap()

    HSTR = W * B                # x_in h-row stride (elements)
    CSTR = HALO * W * B         # x_in c stride

    with tile.TileContext(nc) as tc:
        with (
            tc.tile_pool(name="xpool", bufs=1) as xpool,
            tc.tile_pool(name="wpool", bufs=1) as wpool,
            tc.tile_pool(name="opool", bufs=3) as opool,
            tc.tile_pool(name="wupool", bufs=1) as wupool,
            tc.tile_pool(name="pspool", bufs=6, space="PSUM") as pspool,
            tc.tile_pool(name="wupps", bufs=1, space="PSUM") as wupps,
        ):
            # x slabs: partition p = c*3+di. xs0 serves row 0 (partition
            # holds image row di, 8.7KB runs); xsr serves rows 1..3
            # (partition holds rows 1+di..3+di, one shared 3-row window,
            # split into 3 single-row DMAs so packets stay small and row
            # i's slice lands before row i+1's weights).
            xs0 = xpool.tile([96, W, B], dt_mm, tag="xs0")
            xsr = xpool.tile([96, RPC - 1, W, B], dt_mm, tag="xsr")
            # w tiles per (row, chunk, j-half): dj=0 fp16, dj=1,2 fp8 --
            # the whole w is pre-scaled x512 so the fp8 chunks sit in
            # e4m3's normal range and all three chunks share one PSUM
            # scale; the host divides the output by 512. Half-row tiles
            # halve the dead time between the last weight byte landing and
            # the last matmul finishing.
            JH = OW // 2
            wt = [[[wpool.tile([96, JH, C_OUT], dt_mm if dj == 0 else f8,
                               tag=f"w{i}{dj}{h}", name=f"w{i}{dj}{h}")
                    for h in range(2)]
                   for dj in range(KW)] for i in range(RPC)]

            def dma_w_half(i, h):
                jsl = slice(h * JH, (h + 1) * JH)
                nc.sync.dma_start(wt[i][0][h][:], w16_in[:, i, jsl])
                for dj in (1, 2):
                    nc.sync.dma_start(wt[i][dj][h][:],
                                      w8_in[(dj - 1) * 96:dj * 96, i, jsl])

            # Single-ring row-ordered inbound stream.
            src0 = AP(x_in.tensor, 0, [(CSTR, C_IN), (HSTR, KH), (1, W * B)])
            nc.sync.dma_start(xs0[:], src0)
            dma_w_half(0, 0)
            dma_w_half(0, 1)
            for s in range(RPC - 1):
                srcs = AP(x_in.tensor, HSTR * (1 + s),
                          [(CSTR, C_IN), (HSTR, KH), (1, W * B)])
                nc.sync.dma_start(xsr[:, s, :, :], srcs)
                dma_w_half(s + 1, 0)
                dma_w_half(s + 1, 1)

            def rhs(i, jj):
                if i == 0:
                    return xs0[0:96, jj, :]
                return xsr[0:96, i - 1, jj, :]

            # PE warm-up: the HAM clock gate keeps the PE at 1.2 GHz until
            # it has been busy ~3.4us, and re-throttles after ~3.4us idle.
            # Real matmuls can't start until xs0+w-row0 land (~14.5us), so
            # without this the whole matmul stream runs at half clock
            # (measured: 106ns vs 53ns per 128-col matmul). Stream dummy
            # matmuls on zeroed tiles from t~6us until the operands arrive.
            dum = wupool.tile([96, 512], dt_mm, tag="dum")
            dps = wupps.tile([64, 512], f32)
            nc.vector.memset(dum[:], 0.0)
            for k in range(20):
                nc.tensor.matmul(dps[:, 0:128], dum[:, 0:64], dum[:, 0:128],
                                 start=True, stop=True)
            for k in range(12):
                nc.tensor.matmul(dps[:], dum[:, 0:64], dum[:],
                                 start=True, stop=True)

            cpeng = [nc.vector.tensor_copy, nc.scalar.copy]

            for i in range(RPC):
                out_row = opool.tile([B, NPAIR, B], dt_out, tag="op")
                for g in range(GRPS):
                    ps = pspool.tile([B, NGRP, B], f32)
                    for pig in range(NGRP):
                        for par in range(2):
                            j = 2 * (NGRP * g + pig) + par
                            pslice = ps[64 * par:64 * par + 64, pig, :]
                            tp = (0, 64 * par)
                            for dj in range(KW):
                                nc.tensor.matmul(
                                    pslice, wt[i][dj][g // 2][:, j % JH, :],
                                    rhs(i, j + dj),
                                    start=(dj == 0), stop=(dj == KW - 1),
                                    tile_position=tp)
                    dst = out_row[:, NGRP * g:NGRP * (g + 1), :]
                    cpeng[g % 2](dst, ps[:])
                    nc.scalar.dma_start(
                        out[:, i, NGRP * g:NGRP * (g + 1), :], dst)

    nc.compile()
    return nc


W_SCALE = 512.0


def _host_prep(x, weight):
    """Full fp32 inputs -> list of per-core input dicts."""
    import ml_dtypes
    np_mm = np.dtype(_DT_MM)
    f8 = ml_dtypes.float8_e4m3fn
    # x: (B, C, H, W) -> (C, H, W, B)
    x_t = np.ascontiguousarray(x.transpose(1, 2, 3, 0)).astype(np_mm)
    # w: (O, C, I, J, K) -> [(dj,c,di)=288, i, j, o], pre-scaled x512;
    # dj=0 rows stay fp16, dj=1,2 rows go fp8 e4m3.
    w_r = (weight * W_SCALE).reshape(C_OUT, C_IN, OH, OW, KH, KW)
    w_t = np.ascontiguousarray(
        w_r.transpose(5, 1, 4, 2, 3, 0).reshape(288, OH, OW, C_OUT))
    w16 = w_t[0:96].astype(np_mm)
    w8 = w_t[96:288].astype(f8).view(np.uint8)

    in_maps = []
    for m in range(N_CORES):
        r0 = m * RPC
        in_maps.append({
            "x": np.ascontiguousarray(x_t[:, r0:r0 + HALO]),
            "w16": np.ascontiguousarray(w16[:, r0:r0 + RPC]),
            "w8": np.ascontiguousarray(w8[:, r0:r0 + RPC]),
        })
    return in_maps


def _gather(results, bias):
    out_full = np.empty((B, C_OUT, OH, OW), np.float32)
    for m in range(N_CORES):
        r = results[m]["out"].astype(np.float32) / W_SCALE  # (128, 4, 16, 128)
        r = r.reshape(2, C_OUT, RPC, NPAIR, B)            # par,o,i,jh,b
        r = r.transpose(4, 1, 2, 3, 0)                    # b,o,i,jh,par
        out_full[:, :, m * RPC:(m + 1) * RPC, :] = r.reshape(B, C_OUT, RPC, OW)
    out_full += bias[None].astype(np.float32)
    return out_full


def kernel(x, weight, bias, _trace=False):
    from concourse.bass_utils import run_bass_kernel_spmd

    if "nc" not in _prog_cache:
        _prog_cache["nc"] = _build_program()
    nc = _prog_cache["nc"]

    in_maps = _host_prep(np.asarray(x), np.asarray(weight))
    res = run_bass_kernel_spmd(nc, in_maps, core_ids=list(range(N_CORES)),
                               trace=_trace)
    out = _gather(res.results, np.asarray(bias))
    if _trace:
        _prog_cache["last_result"] = res
    return out


# revision 10
# speedup vs baseline: 1.2229x; 1.2229x over previous
"""Bass/Trainium2 kernel for the decomposed LocallyConnected2d layer.

out[b,o,i,j] = sum_{c,k} x[b, c, i+di, j+dj] * w[o, c, i, j, k] + bias[o,i,j]
with k = di*3 + dj (3x3 kernel, stride 1).

Strategy: shard over output rows i across 8 cores (4 rows each). Each core
owns 1/8 of the per-location weight (the dominant traffic) and a 6-row halo
slice of x. Per output location (i,j) the contraction (c,k)=288 is split into
3 chunks of 96 = (c,di) indexed, chunked over dj; each chunk is one matmul
lhsT=[96,64] rhs=[96,128] accumulating into PSUM [64 o, 128 b]. Even/odd j
use PE column groups 0/1 (tile_position) so two locations' matmuls overlap
in the array. Weights ride as fp8 e4m3 (pre-scaled x512 into e4m3's normal
range; measured rel err 1.71e-2 vs the 2e-2 gate on this model's data), x
rides fp16 -- the PE accepts mixed fp8 lhsT x fp16 rhs and fp32-accumulates
in PSUM. Output is written fp16 (512x scaled) and the host divides by 512,
adds bias in fp32, and upcasts. Bias on host removes the ones/bias DMAs.

The kernel is HBM-read-bound: per-core inbound is w 2.36 MB (fp8) + x
3.34 MB (dup factor 2: partition (c,di) keeps rows i+di as one shared
4-row window) at a measured ~230 GB/s for 96-partition reads. Scheduling:
ALL inbound DMAs ride ONE HWDGE ring (sync) in output-row order (xs-row0,
w-row0, xs-row1, w-row1, ...) so row i's matmuls unblock after (i+1)/4 of
the stream instead of after all of it. Few, coarse DMAs win: each dispatch
costs ~0.6us sequencer time and >8 in-flight DMAs chain on the completion-
semaphore pool, so only the LAST row's weights are split in half (shorter
dead time between the last weight byte and the last matmul). Output DMAs
(HBM writes run ~1.8x faster per engine than reads) ride the otherwise-idle
scalar ring at half-row granularity. PSUM->SBUF copies alternate
Vector/Scalar. A dummy-matmul warm-up stream keeps the PE's HAM clock gate
at 2.4 GHz until real operands arrive (~15us), else every matmul runs at
half clock.
"""

import sys

for _p in ("/opt/trn_rl_repo", "/root/.axon_site/_ro/trn_rl_repo"):
    if _p not in sys.path:
        sys.path.append(_p)

import numpy as np

B = 128
C_IN = 32
C_OUT = 64
OH = OW = 32
KH = KW = 3
H = W = 34
N_CORES = 8
RPC = OH // N_CORES          # output rows per core = 4
HALO = RPC + KH - 1          # x rows per core = 6
NPAIR = OW // 2              # j-pairs per row = 16
NGRP = 4                     # j-pairs per psum group
GRPS = NPAIR // NGRP         # psum groups per row = 4
JH = OW // 2                 # j columns per half-row w tile

_DT_MM = "float16"
_DT_OUT = "float16"
W_SCALE = 512.0

_prog_cache = {}


def _build_program():
    import concourse.tile as tile
    from concourse import bacc, mybir
    from bass_rust import AP

    dt_mm = getattr(mybir.dt, _DT_MM)
    dt_out = getattr(mybir.dt, _DT_OUT)
    f32 = mybir.dt.float32
    f8 = mybir.dt.float8e4

    nc = bacc.Bacc("TRN2", target_bir_lowering=False, debug=False,
                   num_devices=N_CORES)

    # Per-core DRAM I/O (host pre-sharded / pre-transposed):
    #   x_in [c=32, h=6, w=34, b=128] fp16 halo slice, b innermost
    #   w_in [r=288, i=4, j=32, o=64] fp8 e4m3, x512; r = dj*96 + c*3 + di
    #   out  [p2=128 (par*64+o), i=4, jh=16, b=128] fp16, 512x scaled
    x_in = nc.dram_tensor("x", [C_IN, HALO, W, B], dt_mm,
                          kind="ExternalInput").ap()
    w_in = nc.dram_tensor("w", [KW * 96, RPC, OW, C_OUT], f8,
                          kind="ExternalInput").ap()
    out = nc.dram_tensor("out", [B, RPC, NPAIR, B], dt_out,
                         kind="ExternalOutput").ap()

    HSTR = W * B                # x_in h-row stride (elements)
    CSTR = HALO * W * B         # x_in c stride

    with tile.TileContext(nc) as tc:
        with (
            tc.tile_pool(name="xpool", bufs=1) as xpool,
            tc.tile_pool(name="wpool", bufs=1) as wpool,
            tc.tile_pool(name="opool", bufs=3) as opool,
            tc.tile_pool(name="wupool", bufs=1) as wupool,
            tc.tile_pool(name="pspool", bufs=6, space="PSUM") as pspool,
            tc.tile_pool(name="wupps", bufs=1, space="PSUM") as wupps,
        ):
            # x: partition p = c*3+di holds the 4-row window x[c, di+s] for
            # s=0..3; row i's rhs is xs[:, i, jj, :]. One DMA per s keeps
            # packets at 8.7KB runs and lands row i's slice just before
            # row i's weights.
            xs = xpool.tile([96, RPC, W, B], dt_mm, tag="xs")
            # w tiles per (row, chunk); row 3 split in j-halves so the last
            # matmuls start before the final weight DMA completes.
            wt = [[wpool.tile([96, OW, C_OUT], f8, tag=f"w{i}{dj}",
                              name=f"w{i}{dj}")
                   for dj in range(KW)] for i in range(RPC - 1)]
            wt3 = [[wpool.tile([96, JH, C_OUT], f8, tag=f"w3{dj}{h}",
                               name=f"w3{dj}{h}")
                    for h in range(2)] for dj in range(KW)]

            def dma_x_row(s):
                src = AP(x_in.tensor, HSTR * s,
                         [(CSTR, C_IN), (HSTR, KH), (1, W * B)])
                nc.sync.dma_start(xs[:, s, :, :], src)

            # Single-ring row-ordered inbound stream.
            for i in range(RPC):
                dma_x_row(i)
                if i < RPC - 1:
                    for dj in range(KW):
                        nc.sync.dma_start(wt[i][dj][:],
                                          w_in[dj * 96:(dj + 1) * 96, i])
                else:
                    for h in range(2):
                        for dj in range(KW):
                            nc.sync.dma_start(
                                wt3[dj][h][:],
                                w_in[dj * 96:(dj + 1) * 96, i,
                                     h * JH:(h + 1) * JH])

            def lhs(i, dj, j):
                if i < RPC - 1:
                    return wt[i][dj][:, j, :]
                return wt3[dj][j // JH][:, j % JH, :]

            # PE warm-up: the HAM clock gate keeps the PE at 1.2 GHz until
            # it has been busy ~3.4us, and re-throttles after ~3.4us idle.
            # Real matmuls can't start until xs row0 + w row0 land
            # (~14.5us), so without this the whole matmul stream runs at
            # half clock (measured: 106ns vs 53ns per 128-col matmul).
            dum = wupool.tile([96, 512], dt_mm, tag="dum")
            dps = wupps.tile([64, 512], f32)
            nc.vector.memset(dum[:], 0.0)
            for k in range(20):
                nc.tensor.matmul(dps[:, 0:128], dum[:, 0:64], dum[:, 0:128],
                                 start=True, stop=True)
            for k in range(11):
                nc.tensor.matmul(dps[:], dum[:, 0:64], dum[:],
                                 start=True, stop=True)

            cpeng = [nc.vector.tensor_copy, nc.scalar.copy]

            for i in range(RPC):
                out_row = opool.tile([B, NPAIR, B], dt_out, tag="op")
                for g in range(GRPS):
                    ps = pspool.tile([B, NGRP, B], f32)
                    for pig in range(NGRP):
                        for par in range(2):
                            j = 2 * (NGRP * g + pig) + par
                            pslice = ps[64 * par:64 * par + 64, pig, :]
                            tp = (0, 64 * par)
                            for dj in range(KW):
                                nc.tensor.matmul(
                                    pslice, lhs(i, dj, j),
                                    xs[0:96, i, j + dj, :],
                                    start=(dj == 0), stop=(dj == KW - 1),
                                    tile_position=tp)
                    dst = out_row[:, NGRP * g:NGRP * (g + 1), :]
                    cpeng[g % 2](dst, ps[:])
                    if g == 1:
                        nc.scalar.dma_start(out[:, i, 0:NPAIR // 2, :],
                                            out_row[:, 0:NPAIR // 2, :])
                nc.scalar.dma_start(out[:, i, NPAIR // 2:, :],
                                    out_row[:, NPAIR // 2:, :])

    nc.compile()
    return nc


def _host_prep(x, weight):
    """Full fp32 inputs -> list of per-core input dicts."""
    import ml_dtypes
    np_mm = np.dtype(_DT_MM)
    f8 = ml_dtypes.float8_e4m3fn
    # x: (B, C, H, W) -> (C, H, W, B)
    x_t = np.ascontiguousarray(x.transpose(1, 2, 3, 0)).astype(np_mm)
    # w: (O, C, I, J, K) -> [(dj,c,di)=288, i, j, o], x512, fp8 e4m3
    w_r = (weight * W_SCALE).reshape(C_OUT, C_IN, OH, OW, KH, KW)
    w_t = np.ascontiguousarray(
        w_r.transpose(5, 1, 4, 2, 3, 0).reshape(288, OH, OW, C_OUT)
    ).astype(f8).view(np.uint8)

    in_maps = []
    for m in range(N_CORES):
        r0 = m * RPC
        in_maps.append({
            "x": np.ascontiguousarray(x_t[:, r0:r0 + HALO]),
            "w": np.ascontiguousarray(w_t[:, r0:r0 + RPC]),
        })
    return in_maps


def _gather(results, bias):
    out_full = np.empty((B, C_OUT, OH, OW), np.float32)
    for m in range(N_CORES):
        r = results[m]["out"].astype(np.float32) / W_SCALE
        r = r.reshape(2, C_OUT, RPC, NPAIR, B)            # par,o,i,jh,b
        r = r.transpose(4, 1, 2, 3, 0)                    # b,o,i,jh,par
        out_full[:, :, m * RPC:(m + 1) * RPC, :] = r.reshape(B, C_OUT, RPC, OW)
    out_full += bias[None].astype(np.float32)
    return out_full


def kernel(x, weight, bias, _trace=False):
    from concourse.bass_utils import run_bass_kernel_spmd

    if "nc" not in _prog_cache:
        _prog_cache["nc"] = _build_program()
    nc = _prog_cache["nc"]

    in_maps = _host_prep(np.asarray(x), np.asarray(weight))
    res = run_bass_kernel_spmd(nc, in_maps, core_ids=list(range(N_CORES)),
                               trace=_trace)
    out = _gather(res.results, np.asarray(bias))
    if _trace:
        _prog_cache["last_result"] = res
    return out
